# revision 30
# baseline (speedup 1.0000x reference)
"""Complex-valued attention kernel for Trainium2, SPMD over 8 NeuronCores.

Problem (hardcoded shapes): B=4, N=2048, E=384, H=6, D=64, complex64.
  qkv = x @ w_qkv^T + b_qkv          (complex)
  q, k = complex RMSNorm over D (eps=1e-6), affine weights qn_w/kn_w
  scores = Re(q @ conj(k)^H) / sqrt(D)
  attn = softmax(scores)  (real), out = attn @ v   -> [B, N, E] complex64

Sharding: core c handles batch b=c//2, heads 3*(c%2)..3*(c%2)+2 (24 head-
batches over 8 cores, 3 each).

v3 device program per core (complex math decomposed to real planes):
  PH1 per 512-token chunk:
    K path (weight-stationary): k^T[2d, n] produced directly by matmuls with
    w tiles stationary and x^T moving; bias added during the psum->sbuf copy
    on the scalar engine (per-partition bias); k stays UNNORMALIZED - its
    RMS-norm scale folds into PH2's exp as a per-partition (kv) scale.
    Sum-of-squares for the k-norm via scalar Square(+bias) then ones-matmuls
    that land token-major norm columns in psum; one batched sqrt+reciprocal
    at the end of PH1.
    Q/V path (x-stationary): out[token, cols]; bias on vector; per-head
    sum-of-squares via fused tensor_tensor_reduce; q scaled by
    1/sqrt(sum_sq+64eps) (folds 1/sqrt(D)); PE-transposed into qpack.
  PH2 per (q-chunk 1024, head): per kv tile: S_raw^T = kpack_t.T @ qpack;
    exp with scale=rk[kv] on scalar -> es bf16 (no max-subtract needed:
    normalized scores bounded by 8); PV accumulates out^T[2d, q] with vpack
    stationary; Z row sums via ones-stationary matmuls.  Finalize:
    reciprocal of Z first (frees z psum), copy out^T to sbuf (frees pv
    psum), gpsimd partition-broadcast of 1/Z, vector multiply, DMA crossbar
    transpose back to token-major, DMA out in bf16 (host upcasts).
"""

import numpy as np

import concourse.bass as bass
import concourse.tile as tile
from concourse import bacc, mybir
from concourse.bass_utils import run_bass_kernel_spmd

B, N, E, H, D = 4, 2048, 384, 6, 64
EPS = 1e-6
HPC = 3            # heads per core
NT = N // 128      # 16 token tiles
KT = E // 128      # 3 contraction tiles
QC = 2             # q chunks of 1024
NCH = 4            # PH1 512-token chunks
F32 = mybir.dt.float32
MMD = mybir.dt.bfloat16   # matmul operand dtype
F8 = mybir.dt.float8e4
AF = mybir.ActivationFunctionType

# tensor_tensor_reduce fails neuronxcc codegen on this toolchain; use
# tensor_mul + reduce_sum instead.
SAFE_SQ = False
SAFE_EXP = False
SAFE_MSK = False
SAFE_TTR = True
FP8PV = False   # fp8 DoubleRow for PV and Z (es e4m3 shifted by e^-3, v e4m3)

_prog_cache = {}


def _widx(p, a, k):
    return (p * 2 + a) * KT + k


def build_program():
    nc = bacc.Bacc(
        "TRN2", target_bir_lowering=False, debug=False, num_devices=8)
    xt_r = nc.declare_dram_parameter("xt_r", [E, N], MMD, isOutput=False)
    xt_i = nc.declare_dram_parameter("xt_i", [E, N], MMD, isOutput=False)
    w_in = nc.declare_dram_parameter("w", [3, 2, E, 384], MMD, isOutput=False)
    b_in = nc.declare_dram_parameter("bias", [3, 128, 384], F32, isOutput=False)
    bk_in = nc.declare_dram_parameter("bk_col", [128, HPC], F32, isOutput=False)
    id_in = nc.declare_dram_parameter("ident", [128, 128], MMD, isOutput=False)
    on_in = nc.declare_dram_parameter("ones", [128, 1], MMD, isOutput=False)
    on8_in = nc.declare_dram_parameter("ones8", [128, 1], F8, isOutput=False)
    out_d = nc.declare_dram_parameter("out", [N, 384], MMD, isOutput=True)

    with tile.TileContext(nc) as tc:
        with tc.tile_pool(name="persist", bufs=1) as pp:
            w_sb = pp.tile([128, 3 * 2 * KT, 384], MMD)
            bias_sb = pp.tile([128, 3, 384], F32)
            bk_sb = pp.tile([128, HPC], F32)
            ident = pp.tile([128, 128], MMD)
            ones_sb = pp.tile([128, 1], MMD)
            ones8_sb = pp.tile([128, 1], F8)

            def dma_w(p):
                for a in range(2):
                    nc.sync.dma_start(
                        out=w_sb[:, p * 2 * KT + a * KT:
                                 p * 2 * KT + (a + 1) * KT],
                        in_=w_in[p, a].rearrange("(k q) c -> q k c", q=128),
                    )
            dma_w(1)
            nc.scalar.dma_start(out=bk_sb, in_=bk_in[:])

            eps_q = pp.tile([128, 1], F32)
            eps_k = pp.tile([128, 1], F32)
            neg3 = pp.tile([128, 1], F32)
            nc.vector.memset(eps_q, 64.0 * EPS)
            nc.vector.memset(eps_k, EPS)
            nc.vector.memset(neg3, -3.0)

            xt_sb = [pp.tile([128, 2, KT, 512], MMD, name=f"xt{c}")
                     for c in range(NCH)]

            def dma_xt(ch):
                cs = slice(ch * 512, (ch + 1) * 512)
                nc.sync.dma_start(
                    out=xt_sb[ch][:, 0],
                    in_=xt_r[:, cs].rearrange("(k q) n -> q k n", q=128),
                )
                nc.sync.dma_start(
                    out=xt_sb[ch][:, 1],
                    in_=xt_i[:, cs].rearrange("(k q) n -> q k n", q=128),
                )
            dma_xt(0)
            dma_w(0)
            dma_xt(1)
            dma_w(2)
            for ch in (2, 3):
                cs = slice(ch * 512, (ch + 1) * 512)
                nc.scalar.dma_start(
                    out=xt_sb[ch][:, 0],
                    in_=xt_r[:, cs].rearrange("(k q) n -> q k n", q=128))
                nc.scalar.dma_start(
                    out=xt_sb[ch][:, 1],
                    in_=xt_i[:, cs].rearrange("(k q) n -> q k n", q=128))
            nc.scalar.dma_start(
                out=bias_sb, in_=b_in[:].rearrange("p q c -> q p c"))
            nc.scalar.dma_start(out=ident, in_=id_in[:])
            nc.scalar.dma_start(out=ones_sb, in_=on_in[:])
            nc.scalar.dma_start(out=ones8_sb, in_=on8_in[:])

            qpack = pp.tile([128, HPC, N], MMD)   # [2d-interleaved, head, n]
            kpack = pp.tile([128, HPC, N], MMD)
            # [token-in-tile, kv-tile, head*128]
            vpack = pp.tile([128, NT, 384], F8 if FP8PV else MMD)
            rk_all = pp.tile([128, NT, HPC], F32)  # 1/rms_k token-major
            final_sb = pp.tile([128, NT, HPC, 128], MMD)

            # ---------------- PH1: QKV + RMS norm + packing ----------------
            with (
                tc.tile_pool(name="pqv", bufs=3, space="PSUM") as pqv,
                tc.tile_pool(name="pkT", bufs=2, space="PSUM") as pkT,
                tc.tile_pool(name="ptr", bufs=2, space="PSUM") as ptr,
                tc.tile_pool(name="pms", bufs=1, space="PSUM") as pms,
                tc.tile_pool(name="pt1", bufs=3) as pt1,
                tc.tile_pool(name="pdef", bufs=8) as pdef,
            ):
                mskps = pms.tile([128, HPC * NT], F32)
                pend = []

                def flush_pending():
                    # deferred ssk matmuls + q transposes of the previous
                    # chunk (their scalar/vector producers are long done, so
                    # the PE never stalls on them)
                    if not pend:
                        return
                    ch0, sqks0, q2s0 = pend.pop()
                    for hh in range(HPC):
                        for j in range(4):
                            nt = ch0 * 4 + j
                            col = nt * HPC + hh
                            first = (ch0 == 0 and hh == 0 and j == 0)
                            nc.tensor.matmul(
                                mskps[:, col:col + 1],
                                sqks0[hh][:, j * 128:(j + 1) * 128],
                                ones_sb, start=first, stop=True,
                                skip_group_check=True)
                    for j in range(4):
                        nt = ch0 * 4 + j
                        for hh in range(HPC):
                            blk = slice(hh * 128, (hh + 1) * 128)
                            tq = ptr.tile([128, 128], MMD, tag="tr")
                            nc.tensor.transpose(tq, q2s0[j][:, blk], ident)
                            nc.scalar.copy(
                                qpack[:, hh, nt * 128:(nt + 1) * 128], tq)
                    # rk for this chunk: 1/sqrt(sum_sq/64 + eps)
                    s8k = pt1.tile([128, 4 * HPC], F32, tag="s8k")
                    nc.scalar.activation(
                        s8k, mskps[:, ch0 * 4 * HPC:(ch0 + 1) * 4 * HPC],
                        AF.Sqrt, bias=eps_k, scale=1.0 / 64.0)
                    nc.vector.reciprocal(
                        rk_all[:, ch0 * 4:(ch0 + 1) * 4], s8k)

                for ch in range(NCH):
                    cs = slice(ch * 512, (ch + 1) * 512)
                    # --- K path: w-stationary -> kpack [2d, n] directly ---
                    sqks = []
                    for hh in range(HPC):
                        hb = slice(hh * 128, (hh + 1) * 128)
                        pskT = pkT.tile([128, 512], F32, tag="kT")
                        for k in range(KT):
                            for a in range(2):
                                nc.tensor.matmul(
                                    pskT,
                                    w_sb[:, _widx(1, a, k), hb],
                                    xt_sb[ch][:, a, k],
                                    start=(k == 0 and a == 0),
                                    stop=(k == KT - 1 and a == 1))
                        nc.scalar.activation(
                            kpack[:, hh, cs], pskT, AF.Identity,
                            bias=bk_sb[:, hh:hh + 1])
                        sqk = pdef.tile([128, 512], MMD, tag="sqk")
                        if SAFE_SQ:
                            nc.scalar.activation(
                                sqk, kpack[:, hh, cs], AF.Square)
                        else:
                            nc.scalar.activation(
                                sqk, pskT, AF.Square,
                                bias=bk_sb[:, hh:hh + 1])
                        sqks.append(sqk)
                    # --- Q/V path: x-stationary [token, cols] ---
                    q2s_l = []
                    for j in range(4):
                        nt = ch * 4 + j
                        psq = pqv.tile([128, 384], F32, tag="ps")
                        psv = pqv.tile([128, 384], F32, tag="ps")
                        for k in range(KT):
                            for a in range(2):
                                lhs = xt_sb[ch][:, a, k,
                                              j * 128:(j + 1) * 128]
                                st = (k == 0 and a == 0)
                                sp = (k == KT - 1 and a == 1)
                                nc.tensor.matmul(psq, lhs,
                                                 w_sb[:, _widx(0, a, k)],
                                                 start=st, stop=sp)
                                nc.tensor.matmul(psv, lhs,
                                                 w_sb[:, _widx(2, a, k)],
                                                 start=st, stop=sp)
                        q2 = pt1.tile([128, 384], MMD, tag="q2")
                        nc.vector.tensor_add(q2, psq, bias_sb[:, 0])
                        nc.vector.tensor_add(vpack[:, nt], psv, bias_sb[:, 2])
                        msq = pt1.tile([128, HPC], F32, tag="msq")
                        junk = pt1.tile([128, 128], MMD, tag="junk")
                        if SAFE_TTR:
                            sqq = pt1.tile([128, 384], F32, tag="sqq")
                            nc.vector.tensor_mul(sqq, q2, q2)
                            for hh in range(HPC):
                                blk = slice(hh * 128, (hh + 1) * 128)
                                nc.vector.reduce_sum(
                                    msq[:, hh:hh + 1], sqq[:, blk],
                                    axis=mybir.AxisListType.X)
                        else:
                            for hh in range(HPC):
                                blk = slice(hh * 128, (hh + 1) * 128)
                                nc.vector.tensor_tensor_reduce(
                                    junk, q2[:, blk], q2[:, blk], 1.0, 0.0,
                                    mybir.AluOpType.mult, mybir.AluOpType.add,
                                    msq[:, hh:hh + 1])
                        # q scale: 1/(8 sqrt(ms+eps)) = 1/sqrt(sum_sq+64eps)
                        s8q = pt1.tile([128, HPC], F32, tag="s8q")
                        nc.scalar.activation(s8q, msq, AF.Sqrt,
                                             bias=eps_q, scale=1.0)
                        rq = pt1.tile([128, HPC], F32, tag="rq")
                        nc.vector.reciprocal(rq, s8q)
                        q2n = pdef.tile([128, 384], MMD, tag="q2n")
                        for hh in range(HPC):
                            blk = slice(hh * 128, (hh + 1) * 128)
                            nc.vector.tensor_scalar_mul(
                                q2n[:, blk], q2[:, blk], rq[:, hh:hh + 1])
                        q2s_l.append(q2n)
                    # k-norm matmuls + q transposes run one chunk behind
                    flush_pending()
                    pend.append((ch, sqks, q2s_l))
                flush_pending()

            # ---------------- PH2: attention ----------------
            with (
                tc.tile_pool(name="pst", bufs=2, space="PSUM") as pst,
                tc.tile_pool(name="ppv", bufs=1, space="PSUM") as ppv,
                tc.tile_pool(name="pzp", bufs=1, space="PSUM") as pzp,
                tc.tile_pool(name="pes", bufs=3) as pes,
                tc.tile_pool(name="pfo", bufs=2) as pfo,
            ):
                for qc in range(QC):
                    q0 = qc * 1024
                    for hh in range(HPC):
                        pv_ps = ppv.tile([128, 1024], F32, tag="pv")
                        zps = pzp.tile([1, 1024], F32, tag="z")

                        def s_mm(kt, es, slot):
                            st_ps = pst.tile([128, 1024], F32, tag="st",
                                             name="st_ps")
                            for hf in range(2):
                                nc.tensor.matmul(
                                    st_ps[:, hf * 512:(hf + 1) * 512],
                                    kpack[:, hh, kt * 128:(kt + 1) * 128],
                                    qpack[:, hh, q0 + hf * 512:
                                          q0 + (hf + 1) * 512],
                                    start=True, stop=True)
                            # exp(s - 3): e^-3 shift keeps es within e4m3
                            # range (<= e^5 = 148 < 240); cancels in softmax
                            nc.scalar.activation(
                                es[:, slot] if FP8PV else es, st_ps, AF.Exp,
                                scale=rk_all[:, kt, hh:hh + 1],
                                bias=neg3[:, 0:1] if FP8PV else 0.0)

                        if FP8PV:
                            NP = NT // 2
                            DR = mybir.MatmulPerfMode.DoubleRow

                            def pair(p):
                                es2 = pes.tile([128, 2, 1024], F8, tag="es",
                                               name="es2")
                                s_mm(2 * p, es2, 0)
                                s_mm(2 * p + 1, es2, 1)
                                return es2

                            esq = [pair(0)]
                            for p in range(NP):
                                if p + 1 < NP:
                                    esq.append(pair(p + 1))
                                es2 = esq.pop(0)
                                for hf in range(2):
                                    esl = es2[:, :, hf * 512:(hf + 1) * 512]
                                    nc.tensor.matmul(
                                        pv_ps[:, hf * 512:(hf + 1) * 512],
                                        vpack[:, 2 * p:2 * p + 2,
                                              hh * 128:(hh + 1) * 128],
                                        esl,
                                        start=(p == 0), stop=(p == NP - 1),
                                        perf_mode=DR)
                                    for i in range(2):
                                        nc.tensor.matmul(
                                            zps[0:1, hf * 512:(hf + 1) * 512],
                                            ones8_sb,
                                            esl[:, i],
                                            start=(p == 0 and i == 0),
                                            stop=(p == NP - 1 and i == 1))
                        else:
                            def s_exp(kt):
                                es = pes.tile([128, 1024], MMD, tag="es",
                                              name="es")
                                s_mm(kt, es, None)
                                return es

                            es_q = [s_exp(0)]
                            for kt in range(NT):
                                if kt + 1 < NT:
                                    es_q.append(s_exp(kt + 1))
                                es = es_q.pop(0)
                                for hf in range(2):
                                    esl = es[:, hf * 512:(hf + 1) * 512]
                                    nc.tensor.matmul(
                                        pv_ps[:, hf * 512:(hf + 1) * 512],
                                        vpack[:, kt, hh * 128:(hh + 1) * 128],
                                        esl,
                                        start=(kt == 0), stop=(kt == NT - 1))
                                    nc.tensor.matmul(
                                        zps[0:1, hf * 512:(hf + 1) * 512],
                                        ones_sb,
                                        esl,
                                        start=(kt == 0), stop=(kt == NT - 1))
                        # finalize: gpsimd broadcast frees z psum while the
                        # vector copy frees pv psum (parallel); then wide
                        # reciprocal + multiply, one 3D DMA crossbar transpose
                        zr = pfo.tile([1, 1024], F32, tag="zr")
                        nc.vector.reciprocal_approx_fast(zr, zps)
                        outT = pfo.tile([128, 1024], MMD, tag="outT")
                        nc.vector.tensor_copy(outT, pv_ps)
                        zrb = pfo.tile([128, 1024], F32, tag="zrb")
                        nc.gpsimd.partition_broadcast(zrb, zr)
                        outN = pfo.tile([128, 1024], MMD, tag="outN")
                        nc.vector.tensor_mul(outN, outT, zrb)
                        nc.sync.dma_start_transpose(
                            final_sb[:, qc * 8:(qc + 1) * 8, hh], outN)
                        # PH3: output this head's slab right away
                        nc.sync.dma_start(
                            out=out_d[qc * 1024:(qc + 1) * 1024,
                                      hh * 128:(hh + 1) * 128].rearrange(
                                          "(j q) c -> q j c", q=128),
                            in_=final_sb[:, qc * 8:(qc + 1) * 8, hh])
    nc.compile()
    return nc


def _host_prep(x_real, x_imag, w_qkv, b_qkv, qn_w, kn_w):
    """Build the 8 per-core input maps (numpy only)."""
    # fold per-head affine weights into q/k rows.  w row index f = h*D + d
    # within each E block; qn_w has length D (shared across heads).
    qw_col = np.tile(qn_w, H)[:, None]            # [E,1] complex
    kw_col = np.tile(kn_w, H)[:, None]
    wq = w_qkv[0 * E:1 * E] * qw_col
    wk = w_qkv[1 * E:2 * E] * kw_col
    wv = w_qkv[2 * E:3 * E]
    bq = b_qkv[0 * E:1 * E] * qw_col[:, 0]
    bk = b_qkv[1 * E:2 * E] * kw_col[:, 0]
    bv = b_qkv[2 * E:3 * E]

    import ml_dtypes
    bf16 = ml_dtypes.bfloat16
    in_maps = []
    ident = np.eye(128, dtype=bf16)
    ones = np.ones((128, 1), dtype=bf16)
    ones8 = np.ones((128, 1), dtype=ml_dtypes.float8_e4m3)
    for c in range(8):
        b = c // 2
        h0 = HPC * (c % 2)
        # weight tiles: w[pack, plane, e, col] with col = hh*128 + 2d (+1)
        w_arr = np.zeros((3, 2, E, 384), dtype=np.float32)
        b_arr = np.zeros((3, 128, 384), dtype=np.float32)
        bk_col = np.zeros((128, HPC), dtype=np.float32)
        for p, (wm, bm) in enumerate(((wq, bq), (wk, bk), (wv, bv))):
            for hh in range(HPC):
                rows = slice((h0 + hh) * D, (h0 + hh + 1) * D)
                wr = wm[rows].real.T.astype(np.float32)   # [E, D]
                wi = wm[rows].imag.T.astype(np.float32)
                cs = slice(hh * 128, hh * 128 + 128)
                w_arr[p, 0, :, cs.start:cs.stop:2] = wr
                w_arr[p, 0, :, cs.start + 1:cs.stop:2] = wi
                w_arr[p, 1, :, cs.start:cs.stop:2] = -wi
                w_arr[p, 1, :, cs.start + 1:cs.stop:2] = wr
                br = bm[rows].real.astype(np.float32)
                bi = bm[rows].imag.astype(np.float32)
                b_arr[p, :, cs.start:cs.stop:2] = br[None, :]
                b_arr[p, :, cs.start + 1:cs.stop:2] = bi[None, :]
                if p == 1:
                    bk_col[0:128:2, hh] = br
                    bk_col[1:128:2, hh] = bi
        in_maps.append({
            "xt_r": np.ascontiguousarray(x_real[b].T).astype(bf16),
            "xt_i": np.ascontiguousarray(x_imag[b].T).astype(bf16),
            "w": w_arr.astype(bf16),
            "bias": b_arr,
            "bk_col": bk_col,
            "ident": ident,
            "ones": ones,
            "ones8": ones8,
        })
    return in_maps


def _run(x_real, x_imag, w_qkv, b_qkv, qn_w, kn_w, trace=False):
    import time as _t
    if "nc" not in _prog_cache:
        t0 = _t.time()
        _prog_cache["nc"] = build_program()
        print(f"[kernel] program built in {_t.time() - t0:.1f}s", flush=True)
    nc = _prog_cache["nc"]
    t0 = _t.time()
    in_maps = _host_prep(x_real, x_imag, w_qkv, b_qkv, qn_w, kn_w)
    print(f"[kernel] host prep {_t.time() - t0:.1f}s", flush=True)
    t0 = _t.time()
    try:
        res = run_bass_kernel_spmd(nc, in_maps, list(range(8)), trace=trace)
    except Exception as e:
        if not trace:
            raise
        print(f"[kernel] trace run failed ({e!r}); retrying without trace",
              flush=True)
        res = run_bass_kernel_spmd(nc, in_maps, list(range(8)), trace=False)
    print(f"[kernel] device run {_t.time() - t0:.1f}s", flush=True)
    full = np.zeros((B, N, E), dtype=np.complex64)
    for c in range(8):
        b = c // 2
        h0 = HPC * (c % 2)
        oc = res.results[c]["out"].astype(np.float32).view(np.complex64)
        full[b, :, h0 * D:(h0 + HPC) * D] = oc
    return full, res


def kernel(x_real, x_imag, w_qkv, b_qkv, qn_w, kn_w):
    full, _ = _run(x_real, x_imag, w_qkv, b_qkv, qn_w, kn_w, trace=False)
    return full


def kernel_profiled(x_real, x_imag, w_qkv, b_qkv, qn_w, kn_w):
    return _run(x_real, x_imag, w_qkv, b_qkv, qn_w, kn_w, trace=True)


# revision 31
# speedup vs baseline: 1.2208x; 1.2208x over previous
"""Complex-valued attention kernel for Trainium2, SPMD over 8 NeuronCores.

Problem (hardcoded shapes): B=4, N=2048, E=384, H=6, D=64, complex64.
  qkv = x @ w_qkv^T + b_qkv          (complex)
  q, k = complex RMSNorm over D (eps=1e-6), affine weights qn_w/kn_w
  scores = Re(q @ conj(k)^H) / sqrt(D)
  attn = softmax(scores)  (real), out = attn @ v   -> [B, N, E] complex64

Sharding: core c handles batch b=c//2, heads 3*(c%2)..3*(c%2)+2 (24 head-
batches over 8 cores, 3 each).

v3 device program per core (complex math decomposed to real planes):
  PH1 per 512-token chunk:
    K path (weight-stationary): k^T[2d, n] produced directly by matmuls with
    w tiles stationary and x^T moving; bias added during the psum->sbuf copy
    on the scalar engine (per-partition bias); k stays UNNORMALIZED - its
    RMS-norm scale folds into PH2's exp as a per-partition (kv) scale.
    Sum-of-squares for the k-norm via scalar Square(+bias) then ones-matmuls
    that land token-major norm columns in psum; one batched sqrt+reciprocal
    at the end of PH1.
    Q/V path (x-stationary): out[token, cols]; bias on vector; per-head
    sum-of-squares via fused tensor_tensor_reduce; q scaled by
    1/sqrt(sum_sq+64eps) (folds 1/sqrt(D)); PE-transposed into qpack.
  PH2 per (q-chunk 1024, head): per kv tile: S_raw^T = kpack_t.T @ qpack;
    exp with scale=rk[kv] on scalar -> es bf16 (no max-subtract needed:
    normalized scores bounded by 8); PV accumulates out^T[2d, q] with vpack
    stationary; Z row sums via ones-stationary matmuls.  Finalize:
    reciprocal of Z first (frees z psum), copy out^T to sbuf (frees pv
    psum), gpsimd partition-broadcast of 1/Z, vector multiply, DMA crossbar
    transpose back to token-major, DMA out in bf16 (host upcasts).
"""

import numpy as np

import concourse.bass as bass
import concourse.tile as tile
from concourse import bacc, mybir
from concourse.bass_utils import run_bass_kernel_spmd

B, N, E, H, D = 4, 2048, 384, 6, 64
EPS = 1e-6
HPC = 3            # heads per core
NT = N // 128      # 16 token tiles
KT = E // 128      # 3 contraction tiles
QC = 2             # q chunks of 1024
NCH = 4            # PH1 512-token chunks
F32 = mybir.dt.float32
MMD = mybir.dt.bfloat16   # matmul operand dtype
F8 = mybir.dt.float8e4
AF = mybir.ActivationFunctionType

# tensor_tensor_reduce fails neuronxcc codegen on this toolchain; use
# tensor_mul + reduce_sum instead.
SAFE_SQ = False
SAFE_EXP = False
SAFE_MSK = False
SAFE_TTR = True
FP8PV = False   # fp8 DoubleRow for PV and Z (es e4m3 shifted by e^-3, v e4m3)

_prog_cache = {}


def _widx(p, a, k):
    return (p * 2 + a) * KT + k


def build_program():
    nc = bacc.Bacc(
        "TRN2", target_bir_lowering=False, debug=False, num_devices=8)
    xt_r = nc.declare_dram_parameter("xt_r", [E, N], MMD, isOutput=False)
    xt_i = nc.declare_dram_parameter("xt_i", [E, N], MMD, isOutput=False)
    w_in = nc.declare_dram_parameter("w", [3, 2, E, 384], MMD, isOutput=False)
    b_in = nc.declare_dram_parameter("bias", [3, 128, 384], F32, isOutput=False)
    bk_in = nc.declare_dram_parameter("bk_col", [128, HPC], F32, isOutput=False)
    id_in = nc.declare_dram_parameter("ident", [128, 128], MMD, isOutput=False)
    on_in = nc.declare_dram_parameter("ones", [128, 1], MMD, isOutput=False)
    on8_in = nc.declare_dram_parameter("ones8", [128, 1], F8, isOutput=False)
    out_d = nc.declare_dram_parameter("out", [N, 384], MMD, isOutput=True)

    with tile.TileContext(nc) as tc:
        with tc.tile_pool(name="persist", bufs=1) as pp:
            w_sb = pp.tile([128, 3 * 2 * KT, 384], MMD)
            bias_sb = pp.tile([128, 3, 384], F32)
            bk_sb = pp.tile([128, HPC], F32)
            ident = pp.tile([128, 128], MMD)
            ones_sb = pp.tile([128, 1], MMD)
            ones8_sb = pp.tile([128, 1], F8)

            def dma_w(p):
                for a in range(2):
                    nc.sync.dma_start(
                        out=w_sb[:, p * 2 * KT + a * KT:
                                 p * 2 * KT + (a + 1) * KT],
                        in_=w_in[p, a].rearrange("(k q) c -> q k c", q=128),
                    )
            dma_w(1)
            nc.sync.dma_start(out=bk_sb, in_=bk_in[:])

            eps_q = pp.tile([128, 1], F32)
            eps_k = pp.tile([128, 1], F32)
            neg3 = pp.tile([128, 1], F32)
            nc.vector.memset(eps_q, 64.0 * EPS)
            nc.vector.memset(eps_k, EPS)
            nc.vector.memset(neg3, -3.0)

            xt_sb = [pp.tile([128, 2, KT, 512], MMD, name=f"xt{c}")
                     for c in range(NCH)]

            def dma_xt(ch):
                cs = slice(ch * 512, (ch + 1) * 512)
                nc.sync.dma_start(
                    out=xt_sb[ch][:, 0],
                    in_=xt_r[:, cs].rearrange("(k q) n -> q k n", q=128),
                )
                nc.sync.dma_start(
                    out=xt_sb[ch][:, 1],
                    in_=xt_i[:, cs].rearrange("(k q) n -> q k n", q=128),
                )
            dma_xt(0)
            dma_w(0)
            dma_xt(1)
            dma_w(2)
            nc.sync.dma_start(
                out=bias_sb, in_=b_in[:].rearrange("p q c -> q p c"))
            nc.sync.dma_start(out=ident, in_=id_in[:])
            nc.sync.dma_start(out=ones_sb, in_=on_in[:])
            nc.sync.dma_start(out=ones8_sb, in_=on8_in[:])
            dma_xt(2)
            dma_xt(3)

            qpack = pp.tile([128, HPC, N], MMD)   # [2d-interleaved, head, n]
            kpack = pp.tile([128, HPC, N], MMD)
            # [token-in-tile, kv-tile, head*128]
            vpack = pp.tile([128, NT, 384], F8 if FP8PV else MMD)
            rk_all = pp.tile([128, NT, HPC], F32)  # 1/rms_k token-major
            final_sb = pp.tile([128, NT, HPC, 128], MMD)

            # ---------------- PH1: QKV + RMS norm + packing ----------------
            with (
                tc.tile_pool(name="pqv", bufs=3, space="PSUM") as pqv,
                tc.tile_pool(name="pkT", bufs=2, space="PSUM") as pkT,
                tc.tile_pool(name="ptr", bufs=2, space="PSUM") as ptr,
                tc.tile_pool(name="pms", bufs=1, space="PSUM") as pms,
                tc.tile_pool(name="pt1", bufs=3) as pt1,
                tc.tile_pool(name="pdef", bufs=8) as pdef,
            ):
                mskps = pms.tile([128, HPC * NT], F32)
                pend = []

                def flush_pending():
                    # deferred ssk matmuls + q transposes of the previous
                    # chunk (their scalar/vector producers are long done, so
                    # the PE never stalls on them)
                    if not pend:
                        return
                    ch0, sqks0, q2s0 = pend.pop()
                    for hh in range(HPC):
                        for j in range(4):
                            nt = ch0 * 4 + j
                            col = nt * HPC + hh
                            first = (ch0 == 0 and hh == 0 and j == 0)
                            nc.tensor.matmul(
                                mskps[:, col:col + 1],
                                sqks0[hh][:, j * 128:(j + 1) * 128],
                                ones_sb, start=first, stop=True,
                                skip_group_check=True)
                    for j in range(4):
                        nt = ch0 * 4 + j
                        for hh in range(HPC):
                            blk = slice(hh * 128, (hh + 1) * 128)
                            tq = ptr.tile([128, 128], MMD, tag="tr")
                            nc.tensor.transpose(tq, q2s0[j][:, blk], ident)
                            nc.scalar.copy(
                                qpack[:, hh, nt * 128:(nt + 1) * 128], tq)
                    # rk for this chunk: 1/sqrt(sum_sq/64 + eps)
                    s8k = pt1.tile([128, 4 * HPC], F32, tag="s8k")
                    nc.scalar.activation(
                        s8k, mskps[:, ch0 * 4 * HPC:(ch0 + 1) * 4 * HPC],
                        AF.Sqrt, bias=eps_k, scale=1.0 / 64.0)
                    nc.vector.reciprocal(
                        rk_all[:, ch0 * 4:(ch0 + 1) * 4], s8k)

                for ch in range(NCH):
                    cs = slice(ch * 512, (ch + 1) * 512)
                    # --- K path: w-stationary -> kpack [2d, n] directly ---
                    sqks = []
                    for hh in range(HPC):
                        hb = slice(hh * 128, (hh + 1) * 128)
                        pskT = pkT.tile([128, 512], F32, tag="kT")
                        for k in range(KT):
                            for a in range(2):
                                nc.tensor.matmul(
                                    pskT,
                                    w_sb[:, _widx(1, a, k), hb],
                                    xt_sb[ch][:, a, k],
                                    start=(k == 0 and a == 0),
                                    stop=(k == KT - 1 and a == 1))
                        nc.scalar.activation(
                            kpack[:, hh, cs], pskT, AF.Identity,
                            bias=bk_sb[:, hh:hh + 1])
                        sqk = pdef.tile([128, 512], MMD, tag="sqk")
                        if SAFE_SQ:
                            nc.scalar.activation(
                                sqk, kpack[:, hh, cs], AF.Square)
                        else:
                            nc.scalar.activation(
                                sqk, pskT, AF.Square,
                                bias=bk_sb[:, hh:hh + 1])
                        sqks.append(sqk)
                    # --- Q/V path: x-stationary [token, cols] ---
                    q2s_l = []
                    for j in range(4):
                        nt = ch * 4 + j
                        psq = pqv.tile([128, 384], F32, tag="ps")
                        psv = pqv.tile([128, 384], F32, tag="ps")
                        for k in range(KT):
                            for a in range(2):
                                lhs = xt_sb[ch][:, a, k,
                                              j * 128:(j + 1) * 128]
                                st = (k == 0 and a == 0)
                                sp = (k == KT - 1 and a == 1)
                                nc.tensor.matmul(psq, lhs,
                                                 w_sb[:, _widx(0, a, k)],
                                                 start=st, stop=sp)
                                nc.tensor.matmul(psv, lhs,
                                                 w_sb[:, _widx(2, a, k)],
                                                 start=st, stop=sp)
                        q2 = pt1.tile([128, 384], MMD, tag="q2")
                        nc.vector.tensor_add(q2, psq, bias_sb[:, 0])
                        nc.vector.tensor_add(vpack[:, nt], psv, bias_sb[:, 2])
                        msq = pt1.tile([128, HPC], F32, tag="msq")
                        junk = pt1.tile([128, 128], MMD, tag="junk")
                        if SAFE_TTR:
                            sqq = pt1.tile([128, 384], F32, tag="sqq")
                            nc.vector.tensor_mul(sqq, q2, q2)
                            for hh in range(HPC):
                                blk = slice(hh * 128, (hh + 1) * 128)
                                nc.vector.reduce_sum(
                                    msq[:, hh:hh + 1], sqq[:, blk],
                                    axis=mybir.AxisListType.X)
                        else:
                            for hh in range(HPC):
                                blk = slice(hh * 128, (hh + 1) * 128)
                                nc.vector.tensor_tensor_reduce(
                                    junk, q2[:, blk], q2[:, blk], 1.0, 0.0,
                                    mybir.AluOpType.mult, mybir.AluOpType.add,
                                    msq[:, hh:hh + 1])
                        # q scale: 1/(8 sqrt(ms+eps)) = 1/sqrt(sum_sq+64eps)
                        s8q = pt1.tile([128, HPC], F32, tag="s8q")
                        nc.scalar.activation(s8q, msq, AF.Sqrt,
                                             bias=eps_q, scale=1.0)
                        rq = pt1.tile([128, HPC], F32, tag="rq")
                        nc.vector.reciprocal(rq, s8q)
                        q2n = pdef.tile([128, 384], MMD, tag="q2n")
                        for hh in range(HPC):
                            blk = slice(hh * 128, (hh + 1) * 128)
                            nc.vector.tensor_scalar_mul(
                                q2n[:, blk], q2[:, blk], rq[:, hh:hh + 1])
                        q2s_l.append(q2n)
                    # k-norm matmuls + q transposes run one chunk behind
                    flush_pending()
                    pend.append((ch, sqks, q2s_l))
                flush_pending()

            # ---------------- PH2: attention ----------------
            with (
                tc.tile_pool(name="pst", bufs=2, space="PSUM") as pst,
                tc.tile_pool(name="ppv", bufs=1, space="PSUM") as ppv,
                tc.tile_pool(name="pzp", bufs=1, space="PSUM") as pzp,
                tc.tile_pool(name="pes", bufs=3) as pes,
                tc.tile_pool(name="pfo", bufs=2) as pfo,
            ):
                for qc in range(QC):
                    q0 = qc * 1024
                    for hh in range(HPC):
                        pv_ps = ppv.tile([128, 1024], F32, tag="pv")
                        zps = pzp.tile([1, 1024], F32, tag="z")

                        def s_mm(kt, es, slot):
                            st_ps = pst.tile([128, 1024], F32, tag="st",
                                             name="st_ps")
                            for hf in range(2):
                                nc.tensor.matmul(
                                    st_ps[:, hf * 512:(hf + 1) * 512],
                                    kpack[:, hh, kt * 128:(kt + 1) * 128],
                                    qpack[:, hh, q0 + hf * 512:
                                          q0 + (hf + 1) * 512],
                                    start=True, stop=True)
                            # exp(s - 3): e^-3 shift keeps es within e4m3
                            # range (<= e^5 = 148 < 240); cancels in softmax
                            nc.scalar.activation(
                                es[:, slot] if FP8PV else es, st_ps, AF.Exp,
                                scale=rk_all[:, kt, hh:hh + 1],
                                bias=neg3[:, 0:1] if FP8PV else 0.0)

                        if FP8PV:
                            NP = NT // 2
                            DR = mybir.MatmulPerfMode.DoubleRow

                            def pair(p):
                                es2 = pes.tile([128, 2, 1024], F8, tag="es",
                                               name="es2")
                                s_mm(2 * p, es2, 0)
                                s_mm(2 * p + 1, es2, 1)
                                return es2

                            esq = [pair(0)]
                            for p in range(NP):
                                if p + 1 < NP:
                                    esq.append(pair(p + 1))
                                es2 = esq.pop(0)
                                for hf in range(2):
                                    esl = es2[:, :, hf * 512:(hf + 1) * 512]
                                    nc.tensor.matmul(
                                        pv_ps[:, hf * 512:(hf + 1) * 512],
                                        vpack[:, 2 * p:2 * p + 2,
                                              hh * 128:(hh + 1) * 128],
                                        esl,
                                        start=(p == 0), stop=(p == NP - 1),
                                        perf_mode=DR)
                                    for i in range(2):
                                        nc.tensor.matmul(
                                            zps[0:1, hf * 512:(hf + 1) * 512],
                                            ones8_sb,
                                            esl[:, i],
                                            start=(p == 0 and i == 0),
                                            stop=(p == NP - 1 and i == 1))
                        else:
                            def s_exp(kt):
                                es = pes.tile([128, 1024], MMD, tag="es",
                                              name="es")
                                s_mm(kt, es, None)
                                return es

                            es_q = [s_exp(0)]
                            for kt in range(NT):
                                if kt + 1 < NT:
                                    es_q.append(s_exp(kt + 1))
                                es = es_q.pop(0)
                                for hf in range(2):
                                    esl = es[:, hf * 512:(hf + 1) * 512]
                                    nc.tensor.matmul(
                                        pv_ps[:, hf * 512:(hf + 1) * 512],
                                        vpack[:, kt, hh * 128:(hh + 1) * 128],
                                        esl,
                                        start=(kt == 0), stop=(kt == NT - 1))
                                    nc.tensor.matmul(
                                        zps[0:1, hf * 512:(hf + 1) * 512],
                                        ones_sb,
                                        esl,
                                        start=(kt == 0), stop=(kt == NT - 1))
                        # finalize: gpsimd broadcast frees z psum while the
                        # vector copy frees pv psum (parallel); then wide
                        # reciprocal + multiply, one 3D DMA crossbar transpose
                        zr = pfo.tile([1, 1024], F32, tag="zr")
                        nc.vector.reciprocal_approx_fast(zr, zps)
                        outT = pfo.tile([128, 1024], MMD, tag="outT")
                        nc.vector.tensor_copy(outT, pv_ps)
                        zrb = pfo.tile([128, 1024], F32, tag="zrb")
                        nc.gpsimd.partition_broadcast(zrb, zr)
                        outN = pfo.tile([128, 1024], MMD, tag="outN")
                        nc.vector.tensor_mul(outN, outT, zrb)
                        nc.sync.dma_start_transpose(
                            final_sb[:, qc * 8:(qc + 1) * 8, hh], outN)
                        # PH3: output this head's slab right away
                        nc.sync.dma_start(
                            out=out_d[qc * 1024:(qc + 1) * 1024,
                                      hh * 128:(hh + 1) * 128].rearrange(
                                          "(j q) c -> q j c", q=128),
                            in_=final_sb[:, qc * 8:(qc + 1) * 8, hh])
    nc.compile()
    return nc


def _host_prep(x_real, x_imag, w_qkv, b_qkv, qn_w, kn_w):
    """Build the 8 per-core input maps (numpy only)."""
    # fold per-head affine weights into q/k rows.  w row index f = h*D + d
    # within each E block; qn_w has length D (shared across heads).
    qw_col = np.tile(qn_w, H)[:, None]            # [E,1] complex
    kw_col = np.tile(kn_w, H)[:, None]
    wq = w_qkv[0 * E:1 * E] * qw_col
    wk = w_qkv[1 * E:2 * E] * kw_col
    wv = w_qkv[2 * E:3 * E]
    bq = b_qkv[0 * E:1 * E] * qw_col[:, 0]
    bk = b_qkv[1 * E:2 * E] * kw_col[:, 0]
    bv = b_qkv[2 * E:3 * E]

    import ml_dtypes
    bf16 = ml_dtypes.bfloat16
    in_maps = []
    ident = np.eye(128, dtype=bf16)
    ones = np.ones((128, 1), dtype=bf16)
    ones8 = np.ones((128, 1), dtype=ml_dtypes.float8_e4m3)
    for c in range(8):
        b = c // 2
        h0 = HPC * (c % 2)
        # weight tiles: w[pack, plane, e, col] with col = hh*128 + 2d (+1)
        w_arr = np.zeros((3, 2, E, 384), dtype=np.float32)
        b_arr = np.zeros((3, 128, 384), dtype=np.float32)
        bk_col = np.zeros((128, HPC), dtype=np.float32)
        for p, (wm, bm) in enumerate(((wq, bq), (wk, bk), (wv, bv))):
            for hh in range(HPC):
                rows = slice((h0 + hh) * D, (h0 + hh + 1) * D)
                wr = wm[rows].real.T.astype(np.float32)   # [E, D]
                wi = wm[rows].imag.T.astype(np.float32)
                cs = slice(hh * 128, hh * 128 + 128)
                w_arr[p, 0, :, cs.start:cs.stop:2] = wr
                w_arr[p, 0, :, cs.start + 1:cs.stop:2] = wi
                w_arr[p, 1, :, cs.start:cs.stop:2] = -wi
                w_arr[p, 1, :, cs.start + 1:cs.stop:2] = wr
                br = bm[rows].real.astype(np.float32)
                bi = bm[rows].imag.astype(np.float32)
                b_arr[p, :, cs.start:cs.stop:2] = br[None, :]
                b_arr[p, :, cs.start + 1:cs.stop:2] = bi[None, :]
                if p == 1:
                    bk_col[0:128:2, hh] = br
                    bk_col[1:128:2, hh] = bi
        in_maps.append({
            "xt_r": np.ascontiguousarray(x_real[b].T).astype(bf16),
            "xt_i": np.ascontiguousarray(x_imag[b].T).astype(bf16),
            "w": w_arr.astype(bf16),
            "bias": b_arr,
            "bk_col": bk_col,
            "ident": ident,
            "ones": ones,
            "ones8": ones8,
        })
    return in_maps


def _run(x_real, x_imag, w_qkv, b_qkv, qn_w, kn_w, trace=False):
    import time as _t
    if "nc" not in _prog_cache:
        t0 = _t.time()
        _prog_cache["nc"] = build_program()
        print(f"[kernel] program built in {_t.time() - t0:.1f}s", flush=True)
    nc = _prog_cache["nc"]
    t0 = _t.time()
    in_maps = _host_prep(x_real, x_imag, w_qkv, b_qkv, qn_w, kn_w)
    print(f"[kernel] host prep {_t.time() - t0:.1f}s", flush=True)
    t0 = _t.time()
    try:
        res = run_bass_kernel_spmd(nc, in_maps, list(range(8)), trace=trace)
    except Exception as e:
        if not trace:
            raise
        print(f"[kernel] trace run failed ({e!r}); retrying without trace",
              flush=True)
        res = run_bass_kernel_spmd(nc, in_maps, list(range(8)), trace=False)
    print(f"[kernel] device run {_t.time() - t0:.1f}s", flush=True)
    full = np.zeros((B, N, E), dtype=np.complex64)
    for c in range(8):
        b = c // 2
        h0 = HPC * (c % 2)
        oc = res.results[c]["out"].astype(np.float32).view(np.complex64)
        full[b, :, h0 * D:(h0 + HPC) * D] = oc
    return full, res


def kernel(x_real, x_imag, w_qkv, b_qkv, qn_w, kn_w):
    full, _ = _run(x_real, x_imag, w_qkv, b_qkv, qn_w, kn_w, trace=False)
    return full


def kernel_profiled(x_real, x_imag, w_qkv, b_qkv, qn_w, kn_w):
    return _run(x_real, x_imag, w_qkv, b_qkv, qn_w, kn_w, trace=True)
